# revision 37
# baseline (speedup 1.0000x reference)
"""GNN message-passing convolution on 8 Trainium2 NeuronCores.

Strategy (receiver-sharded, zero collectives, host pre-gather):
  - Host sorts edges by receiver; core k owns receivers [6250k, 6250(k+1)).
  - Each 128-receiver window's edges occupy chunks of 128 slots. Windows
    are assigned to program "slots" sorted by edge count (heaviest window
    of each core -> slot 0, ...), so slot j's chunk count C_j is the max
    over cores of the j-th largest window, instead of one uniform max.
  - Host pre-gathers sender node rows into a dense [P, TC*128] bf16
    stream per core, so the device needs only plain contiguous DMA (no
    gpsimd dma_gather descriptor generation - the v1 bottleneck, ~6us
    per window of Pool-engine time).
  - The one-hot scatter matrices are also host-built and DMA-streamed
    (Pool/TRN2 has no is_equal, and DVE is the busiest engine).
  - Edge-scalar MLP runs "paired": chunks 2q/2q+1 share columns with
    block-diagonal weights, so h0/h1 matmuls cost half the free-dim and
    SILU uses all 128 partitions.
  - Equivariant tensor product + gating on VectorE, window-batched; the
    edge-attr broadcast is materialized once per window on the otherwise
    idle Pool engine so the big multiplies hit the DVE 16-bit 2x mode.
  - Scatter-add via one-hot matmul accumulating into PSUM; scatter for
    slot j-1 is issued after the MLP of slot j (software pipelining) so
    the PE never head-of-line blocks on DVE.
  - Host reorders per-core row blocks back to window order, concatenates,
    and un-permutes columns.
"""

import numpy as np

N_NODES = 50000
N_EDGES = 800000
MUL = 32
NCORES = 8
NODES_PER_CORE = N_NODES // NCORES          # 6250
P = 128
WINDOWS = (NODES_PER_CORE + P - 1) // P     # 49
INV_SQRT3 = 1.0 / np.sqrt(3.0)
AVG_NUM_NEIGHBORS = 16.0

_CACHE = {}


def _col_perms():
    # node table planar permutation: new[32+32*i+c] = old[32+3*c+i]
    node_perm = np.concatenate(
        [np.arange(32)]
        + [np.array([32 + 3 * c + i for c in range(32)]) for i in range(3)]
    )
    # output un-permutation: ref[64+3c+i] = int[64+32i+c]; same at 160
    out_perm = np.empty(256, np.int64)
    out_perm[0:64] = np.arange(64)
    for c in range(32):
        for i in range(3):
            out_perm[64 + 3 * c + i] = 64 + 32 * i + c
            out_perm[160 + 3 * c + i] = 160 + 32 * i + c
    return node_perm, out_perm


def _build_program(slot_Cs, sim_silu=False):
    import concourse.bacc as bacc
    import concourse.mybir as mybir
    import concourse.tile as tile

    f32 = mybir.dt.float32
    bf16 = mybir.dt.bfloat16
    AF = mybir.ActivationFunctionType
    OP = mybir.AluOpType

    slot_Cs = list(slot_Cs)
    TC = sum(slot_Cs)
    Cmax = max(slot_Cs)
    NPmax = Cmax // 2
    off = np.concatenate([[0], np.cumsum(slot_Cs)])

    nc = bacc.Bacc("TRN2", target_bir_lowering=False, debug=False,
                   num_devices=NCORES, num_swdge_queues=4)

    G_d = nc.dram_tensor("Gw", [P, TC * 128], bf16, kind="ExternalInput")
    oh_d = nc.dram_tensor("ohw", [P, TC * 128], bf16, kind="ExternalInput")
    ea0_d = nc.dram_tensor("ea0p", [2, (TC // 2) * P], bf16,
                           kind="ExternalInput")
    earep_d = nc.dram_tensor("earep", [P, TC * 12], bf16,
                             kind="ExternalInput")
    w0_d = nc.dram_tensor("w0bd", [2, 128], bf16, kind="ExternalInput")
    w1_d = nc.dram_tensor("w1bd", [128, 128], bf16, kind="ExternalInput")
    w2_d = nc.dram_tensor("w2bd", [128, 256], bf16, kind="ExternalInput")
    out_d = nc.dram_tensor("out", [WINDOWS * P, 256], bf16,
                           kind="ExternalOutput")

    with tile.TileContext(nc) as tc:
        with (
            tc.tile_pool(name="const", bufs=1) as cp,
            tc.tile_pool(name="gpool", bufs=4) as gp,
            tc.tile_pool(name="sb", bufs=3) as sb,
            tc.tile_pool(name="msp", bufs=3) as msp,
            tc.tile_pool(name="ohp", bufs=3) as ohp,
            tc.tile_pool(name="stage", bufs=3) as stp,
            tc.tile_pool(name="psH", bufs=2, space="PSUM") as psH,
            tc.tile_pool(name="psM", bufs=2, space="PSUM") as psM,
            tc.tile_pool(name="psC", bufs=2, space="PSUM") as psC,
        ):
            # ---- resident constants ----
            w0_t = cp.tile([2, 128], bf16)
            nc.sync.dma_start(out=w0_t[:], in_=w0_d.ap())
            w1_t = cp.tile([128, 128], bf16)
            nc.sync.dma_start(out=w1_t[:], in_=w1_d.ap())
            w2_t = cp.tile([128, 256], bf16)
            nc.sync.dma_start(out=w2_t[:], in_=w2_d.ap())

            state = [None] * WINDOWS

            def emit_front(j):
                C = slot_Cs[j]
                NPAIR = C // 2
                o = int(off[j])

                G = gp.tile([P, Cmax * 128], bf16, tag="G", name=f"G_s{j}")
                nc.sync.dma_start(out=G[:, :C * 128],
                                  in_=G_d.ap()[:, o * 128:(o + C) * 128])
                oh = ohp.tile([P, Cmax * 128], bf16, tag="oh", name=f"oh_s{j}")
                nc.sync.dma_start(out=oh[:, :C * 128],
                                  in_=oh_d.ap()[:, o * 128:(o + C) * 128])
                ea0_t = sb.tile([2, NPmax * P], bf16, tag="ea0",
                                name=f"ea0_s{j}")
                po = (o // 2) * P
                nc.sync.dma_start(out=ea0_t[:, :NPAIR * P],
                                  in_=ea0_d.ap()[:, po:po + NPAIR * P])
                ea_rep = sb.tile([P, Cmax * 12], bf16, tag="ea_rep",
                                 name=f"ea_rep_s{j}", bufs=3)
                nc.sync.dma_start(out=ea_rep[:, :C * 12],
                                  in_=earep_d.ap()[:, o * 12:(o + C) * 12])

                # ---- paired MLP (blocks of <=8 pairs; one SILU per block
                # layer, reading a 2-bank [P, 1024] PSUM tile) ----
                h1_t = sb.tile([P, NPmax * P], bf16, tag="h1", name=f"h1_s{j}")
                q = 0
                while q < NPAIR:
                    nq = min(8, NPAIR - q)
                    c0, cols = q * P, nq * P
                    h0p = psH.tile([P, 1024], f32, tag="hp",
                                   name=f"h0p_{j}_{q}")
                    for s0 in range(0, cols, 512):
                        s1 = min(s0 + 512, cols)
                        nc.tensor.matmul(out=h0p[:, s0:s1], lhsT=w0_t[:, :],
                                         rhs=ea0_t[:, c0 + s0:c0 + s1],
                                         start=True, stop=True)
                    h0 = sb.tile([P, 1024], bf16, tag="h0", name=f"h0_{j}_{q}")
                    if sim_silu:
                        sg0 = sb.tile([P, 1024], f32, tag="sg0",
                                      name=f"sg0_{j}_{q}")
                        nc.scalar.activation(out=sg0[:, :cols],
                                             in_=h0p[:, :cols],
                                             func=AF.Sigmoid)
                        nc.vector.tensor_tensor(out=h0[:, :cols],
                                                in0=sg0[:, :cols],
                                                in1=h0p[:, :cols], op=OP.mult)
                    else:
                        nc.scalar.activation(out=h0[:, :cols],
                                             in_=h0p[:, :cols], func=AF.Silu)
                    h1p = psH.tile([P, 1024], f32, tag="hp",
                                   name=f"h1p_{j}_{q}")
                    for s0 in range(0, cols, 512):
                        s1 = min(s0 + 512, cols)
                        nc.tensor.matmul(out=h1p[:, s0:s1], lhsT=w1_t[:, :],
                                         rhs=h0[:, s0:s1], start=True,
                                         stop=True)
                    if sim_silu:
                        sg1 = sb.tile([P, 1024], f32, tag="sg1",
                                      name=f"sg1_{j}_{q}")
                        nc.scalar.activation(out=sg1[:, :cols],
                                             in_=h1p[:, :cols],
                                             func=AF.Sigmoid, scale=0.125)
                        h1s = sb.tile([P, 1024], f32, tag="h1s",
                                      name=f"h1s_{j}_{q}")
                        nc.scalar.activation(out=h1s[:, :cols],
                                             in_=h1p[:, :cols],
                                             func=AF.Copy, scale=0.125)
                        nc.vector.tensor_tensor(out=h1_t[:, c0:c0 + cols],
                                                in0=sg1[:, :cols],
                                                in1=h1s[:, :cols], op=OP.mult)
                    else:
                        nc.scalar.activation(out=h1_t[:, c0:c0 + cols],
                                             in_=h1p[:, :cols], func=AF.Silu,
                                             scale=0.125)
                    q += nq

                mix_t = sb.tile([P, Cmax * 128], bf16, tag="mix",
                                name=f"mix_s{j}")
                q = 0
                while q < NPAIR:
                    nq = min(2, NPAIR - q)      # two pairs share a PSUM bank
                    mixp = psM.tile([P, 512], f32, tag="mixp",
                                    name=f"mixp_{j}_{q}")
                    for i in range(nq):
                        nc.tensor.matmul(
                            out=mixp[:, i * 256:(i + 1) * 256],
                            lhsT=h1_t[:, (q + i) * P:(q + i + 1) * P],
                            rhs=w2_t[:, :], start=True, stop=True)
                    nc.scalar.activation(
                        out=mix_t[:, q * 256:(q + nq) * 256],
                        in_=mixp[:, :nq * 256], func=AF.Copy)
                    q += nq

                # ---- tensor product + gating (DVE, window-batched) ----
                Gr = G[:, :C * 128].rearrange("p (c f) -> p c f", f=128)
                Gs = Gr[:, :, 0:32]
                Gv = Gr[:, :, 32:128].rearrange("p c (i x) -> p c i x", i=3)
                mixr = mix_t[:, :C * 128].rearrange("p (c f) -> p c f", f=128)

                msgs = msp.tile([P, Cmax, 256], bf16, tag="msgs",
                                name=f"msgs_s{j}")
                # ea1 comes x4-replicated from the host: the innermost
                # stride-1 run of 4 keeps the DVE 16-bit 2x mode on the two
                # big multiplies reading it (a stride-0 innermost operand
                # would disable it), at 1/8 the DMA bytes of full expansion.
                ea_r = ea_rep[:, :C * 12] \
                    .rearrange("p (c i x) -> p c i x", i=3, x=4) \
                    .unsqueeze(3).to_broadcast([P, C, 3, 8, 4])
                Gv5 = Gr[:, :, 32:128] \
                    .rearrange("p c (i y x) -> p c i y x", i=3, y=8, x=4)
                tmp96 = sb.tile([P, Cmax, 3, 32], bf16, tag="tmp96",
                                name=f"tmp96_s{j}")
                t96_5 = tmp96[:, :C, :, :] \
                    .rearrange("p c i (y x) -> p c i y x", y=8, x=4)
                nc.vector.tensor_tensor(out=t96_5, in0=Gv5,
                                        in1=ea_r, op=OP.mult)
                tp0a = sb.tile([P, Cmax, 32], bf16, tag="tp0a",
                               name=f"tp0a_s{j}")
                nc.vector.tensor_tensor(out=tp0a[:, :C, :],
                                        in0=tmp96[:, :C, 0, :],
                                        in1=tmp96[:, :C, 1, :], op=OP.add)
                tp0b = sb.tile([P, Cmax, 32], bf16, tag="tp0b",
                               name=f"tp0b_s{j}")
                nc.vector.tensor_tensor(out=tp0b[:, :C, :],
                                        in0=tp0a[:, :C, :],
                                        in1=tmp96[:, :C, 2, :], op=OP.add)

                nc.vector.tensor_tensor(out=msgs[:, :C, 0:32], in0=Gs,
                                        in1=mixr[:, :, 0:32], op=OP.mult)
                nc.vector.tensor_tensor(out=msgs[:, :C, 32:64],
                                        in0=tp0b[:, :C, :],
                                        in1=mixr[:, :, 32:64], op=OP.mult)
                mix_v = mixr[:, :, 64:96].unsqueeze(2) \
                    .to_broadcast([P, C, 3, 32])
                nc.vector.tensor_tensor(
                    out=msgs[:, :C, 64:160]
                    .rearrange("p c (i x) -> p c i x", i=3),
                    in0=Gv, in1=mix_v, op=OP.mult)
                sg2 = sb.tile([P, Cmax, 32], bf16, tag="sg2", name=f"sg2_s{j}")
                nc.vector.tensor_tensor(out=sg2[:, :C, :], in0=Gs,
                                        in1=mixr[:, :, 96:128], op=OP.mult)
                sg2_b = sg2[:, :C, :] \
                    .rearrange("p c (y x) -> p c y x", y=8, x=4) \
                    .unsqueeze(2).to_broadcast([P, C, 3, 8, 4])
                nc.vector.tensor_tensor(
                    out=msgs[:, :C, 160:256]
                    .rearrange("p c (i y x) -> p c i y x", i=3, y=8, x=4),
                    in0=sg2_b, in1=ea_r, op=OP.mult)

                state[j] = (oh, msgs)

            def emit_scatter(j):
                C = slot_Cs[j]
                oh, msgs = state[j]
                ohr = oh[:, :C * 128].rearrange("p (c f) -> p c f", f=128)
                acc = psC.tile([P, 256], f32, tag="acc", name=f"acc_s{j}")
                for c in range(C):
                    nc.tensor.matmul(out=acc[:, :], lhsT=ohr[:, c, :],
                                     rhs=msgs[:, c, :],
                                     start=(c == 0), stop=(c == C - 1))
                ot = stp.tile([P, 256], bf16, tag="ot", name=f"ot_s{j}")
                nc.vector.tensor_copy(out=ot[:, :], in_=acc[:, :])
                nc.sync.dma_start(out=out_d.ap()[j * P:(j + 1) * P, :],
                                  in_=ot[:, :])
                state[j] = None

            for j in range(WINDOWS):
                emit_front(j)
                if j >= 1:
                    emit_scatter(j - 1)
            emit_scatter(WINDOWS - 1)

    nc.compile()
    return nc


def _prep_inputs(node_feats, edge_attrs, senders, receivers, w_mlp0, w_mlp1,
                 w_mlp2):
    import ml_dtypes
    bf = ml_dtypes.bfloat16

    node_perm, out_perm = _col_perms()

    senders = np.asarray(senders).astype(np.int64)
    receivers = np.asarray(receivers).astype(np.int64)
    edge_attrs = np.asarray(edge_attrs, dtype=np.float32)
    node_feats = np.asarray(node_feats, dtype=np.float32)

    order = np.argsort(receivers, kind="stable")
    r_s = receivers[order]
    s_s = senders[order]
    ea_s = edge_attrs[order]

    bounds = np.searchsorted(r_s, np.arange(NCORES + 1) * NODES_PER_CORE)

    core_data = []
    ords = []
    sorted_cnts = []
    for k in range(NCORES):
        a, b = bounds[k], bounds[k + 1]
        lrcv = r_s[a:b] - k * NODES_PER_CORE
        win = (lrcv >> 7).astype(np.int64)
        cnt = np.bincount(win, minlength=WINDOWS)
        ordk = np.argsort(-cnt, kind="stable")      # slot j -> window id
        ords.append(ordk)
        sorted_cnts.append(cnt[ordk])
        core_data.append((a, b, lrcv, win, cnt))
    slot_max = np.max(np.stack(sorted_cnts), axis=0)     # [WINDOWS]
    slot_C = np.maximum(-(-slot_max // P), 2).astype(np.int64)
    slot_C += slot_C & 1                                  # even per slot
    off = np.concatenate([[0], np.cumsum(slot_C)])
    TC = int(off[-1])

    node_bf = np.ascontiguousarray(node_feats[:, node_perm]).astype(bf)
    w0 = np.asarray(w_mlp0, dtype=np.float32)
    w1 = np.asarray(w_mlp1, dtype=np.float32)
    w2s = (np.asarray(w_mlp2, dtype=np.float32) / 32.0).copy()
    w2s[:, 32:64] *= INV_SQRT3

    w0_bd = np.zeros((2, 128), np.float32)
    w0_bd[0, 0:64] = w0[0]
    w0_bd[1, 64:128] = w0[0]
    w1_bd = np.zeros((128, 128), np.float32)
    w1_bd[0:64, 0:64] = w1
    w1_bd[64:128, 64:128] = w1
    w2_bd = np.zeros((128, 256), np.float32)
    w2_bd[0:64, 0:128] = w2s
    w2_bd[64:128, 128:256] = w2s

    shared = {
        "w0bd": w0_bd.astype(bf),
        "w1bd": w1_bd.astype(bf),
        "w2bd": w2_bd.astype(bf),
    }

    in_maps = []
    for k in range(NCORES):
        a, b, lrcv, win, cnt = core_data[k]
        ordk = ords[k]
        inv = np.empty(WINDOWS, np.int64)
        inv[ordk] = np.arange(WINDOWS)

        win_start = np.r_[0, np.cumsum(cnt)[:-1]]
        rank = np.arange(b - a) - win_start[win]
        j = inv[win]                                  # slot of each edge
        chunk = off[j] + (rank >> 7)
        pslot = rank & 127
        flat = chunk * P + pslot
        rloc = lrcv - (win << 7)

        sp = np.zeros(TC * P, np.int64)
        eap = np.zeros((TC * P, 3), np.float32)
        e0p = np.zeros(TC * P, np.float32)
        sp[flat] = s_s[a:b]
        eap[flat, 0:3] = ea_s[a:b, 1:4]
        e0p[flat] = ea_s[a:b, 0]

        # pre-gathered sender rows: [P, TC*128]
        G = node_bf[sp.reshape(TC, P)]                    # [TC, P, 128]
        G = np.ascontiguousarray(G.transpose(1, 0, 2)).reshape(P, TC * 128)

        # one-hot scatter matrices: [P, TC*128]
        ohb = np.zeros((TC, P, P), bf)
        ohb[chunk, pslot, rloc] = 1.0
        oh = np.ascontiguousarray(ohb.transpose(1, 0, 2)) \
            .reshape(P, TC * 128)

        # ea0 paired layout: [2, (TC//2)*P]
        ea0p = np.ascontiguousarray(
            e0p.reshape(TC // 2, 2, P).transpose(1, 0, 2)).reshape(2, -1)

        # ea1 pre-replicated x4 (innermost stride-1 run for DVE 2x): [P, TC*12]
        ea1p = eap.reshape(TC, P, 3).transpose(1, 0, 2).astype(bf)
        earep = np.ascontiguousarray(
            np.broadcast_to(ea1p[:, :, :, None], (P, TC, 3, 4))) \
            .reshape(P, TC * 12)

        in_maps.append({
            "Gw": G,
            "ohw": oh,
            "ea0p": ea0p.astype(bf),
            "earep": earep,
            **shared,
        })
    return in_maps, tuple(int(c) for c in slot_C), ords, out_perm


def kernel(node_feats, edge_attrs, senders, receivers, w_mlp0, w_mlp1, w_mlp2):
    from concourse import bass_utils

    in_maps, slot_Cs, ords, out_perm = _prep_inputs(
        node_feats, edge_attrs, senders, receivers, w_mlp0, w_mlp1, w_mlp2)

    if slot_Cs not in _CACHE:
        _CACHE[slot_Cs] = _build_program(slot_Cs)
    nc = _CACHE[slot_Cs]

    res = bass_utils.run_bass_kernel_spmd(
        nc, in_maps, core_ids=list(range(NCORES)))

    out = np.empty((N_NODES, 256), np.float32)
    for k in range(NCORES):
        r = np.asarray(res.results[k]["out"], dtype=np.float32)
        ordk = ords[k]
        for j in range(WINDOWS):
            wid = int(ordk[j])
            n = min(P, NODES_PER_CORE - wid * P)
            out[k * NODES_PER_CORE + wid * P:
                k * NODES_PER_CORE + wid * P + n] = r[j * P:j * P + n]
    return np.ascontiguousarray(out[:, out_perm])
